# revision 1
# baseline (speedup 1.0000x reference)
"""Multi-head attention (B=2, T=2048, D=OUT=1024, H=16) on 8 TRN2 NeuronCores.

Sharding: data-parallel over batch (2 groups) x tensor-parallel over heads
(4 groups of 4 heads). Core c handles batch c//4, heads (c%4)*4..(c%4)*4+4.
Each core computes Q^T/K^T/V for its head group, streams softmax(QK^T)V
in transposed layout (keys on partitions), and a partial output projection
through its W_o row block. The host sums the 4 partials per batch and adds
b_o.

Device-side layout notes:
- x is fed transposed ([D, T]) so Q^T/K^T come straight out of the PE.
- The K projection psum is split-written directly into the per-head
  zero-padded kth tiles (no staging copy): head 2mi keeps psum rows 0:64,
  head 2mi+1 keeps rows 64:128, matching where the paired Q^T tile holds
  that head's rows. Every attention matmul then contracts K=128 (K=64
  matmuls do not register as PE activity for the HAM clock gate and run
  at half clock forever).
- The PV stationary is a full 128-column tile per (kt, head) so LDWEIGHTS
  gets fast-weight-load; the softmax denominator rides along as a ones
  column inside it. Per-head column placement is parity-asymmetric so the
  PV output lands directly at the at2p partitions that head occupies:
    even h: V at cols 0:64 (out rows 0:64), ones at col 64 (den at p64)
    odd  h: ones at col 0 (den at p0), V at cols 64:128 (out rows 64:128)
  Unused columns are zeroed once at startup. The odd-head path needs no
  cross-partition DMA hop at all: reciprocal reads the den row straight
  from psum partition 0 and partition_broadcast fills p64:128.
- Matmul operands are bf16 (fp32 PSUM accumulation): full PE clock and
  fast weight load; fp32r measured 2x slower.
- Output partials are written as fp16 ([OUT, T]) -- halves the outbound
  DMA; the host accumulates in fp32 (adds ~1e-4 relative error).
- Emission is one globally software-pipelined stream over all 128
  attention units (PV of unit u-1 emitted inside unit u, across head
  boundaries; attn psum bufs=2 keeps both heads' tiles alive at a
  transition). Filler work (remaining projections, W_o load, first half
  of the output projection) rides on non-transition units. Head order
  within a query block is [1, 0, 2, 3] so the final normalize is an odd
  head (its den needs no DMA hop off p64).
"""

import numpy as np

import concourse.bass as bass
import concourse.mybir as mybir
import concourse.tile as tile
from concourse import bacc
from concourse.bass_utils import run_bass_kernel_spmd

B, T, D, OUT, H = 2, 2048, 1024, 1024, 16
DO = 256            # output columns per core (4 heads x 64)
DEPTH = 64
NH = 4              # heads per core
KT = D // 128       # 8 contraction tiles for the projections
TT = T // 128       # 16 key tiles
NB = T // 512       # 4 query/time blocks
F32 = mybir.dt.float32
F16 = mybir.dt.float16
BF16 = mybir.dt.bfloat16
MMDT = BF16
EXP = mybir.ActivationFunctionType.Exp
MULT = mybir.AluOpType.mult
ADD = mybir.AluOpType.add

H_ORDER = [1, 0, 2, 3]   # emission order of heads within a query block

_CACHE = {}


def build_attention(nc, dbg=None):
    # x is sent chunk-contiguous: [nb, d, c] so each [128, 512] chunk DMA
    # is one contiguous 128KB transfer
    xt = nc.declare_dram_parameter("xt", [NB * D, 512], MMDT, isOutput=False)
    # weights are pre-arranged on the host into the SBUF layout so the
    # DMAs are contiguous full-rate transfers
    wq = nc.declare_dram_parameter("wq", [128, KT * DO], MMDT, isOutput=False)
    wk = nc.declare_dram_parameter("wk", [128, KT * DO], MMDT, isOutput=False)
    wv = nc.declare_dram_parameter("wv", [128, KT * DO], MMDT, isOutput=False)
    wo = nc.declare_dram_parameter("wo", [128, 2 * OUT], MMDT, isOutput=False)
    bq2 = nc.declare_dram_parameter("bq2", [128, 2], F32, isOutput=False)
    bv = nc.declare_dram_parameter("bv", [DO], F32, isOutput=False)
    bcol = nc.declare_dram_parameter("bcol", [128, TT], F32, isOutput=False)
    # output partials are chunk-contiguous, twice (one per head-pair j):
    # [j, tb, nt-rows, 512]; the host sums the two j-partials
    outT = nc.declare_dram_parameter("outT", [2 * NB * OUT, 512], F16, isOutput=True)

    with tile.TileContext(nc) as tc:
        with (
            tc.tile_pool(name="cw", bufs=1) as cw,
            tc.tile_pool(name="stage", bufs=8) as stage,
            tc.tile_pool(name="persist", bufs=1) as persist,
            tc.tile_pool(name="small", bufs=2) as small,
            tc.tile_pool(name="ptp", bufs=8) as ptp,
            tc.tile_pool(name="px", bufs=1) as px,
            tc.tile_pool(name="ps_s", bufs=2, space="PSUM") as ps_s,
            tc.tile_pool(name="ps_mm", bufs=2, space="PSUM") as ps_mm,
        ):
            # ---- warmup: wake the PE HAM clock gate and the ACT exp table
            # while the input DMAs are in flight ----
            ones_f = cw.tile([128, NH], F32, tag="ones")
            nc.vector.memset(ones_f[:], 1.0)
            warm_w = cw.tile([128, 256], MMDT, tag="warmw")
            nc.vector.memset(warm_w[:], 0.0)
            warm_ps = ps_s.tile([128, 256], F32, tag="s", name="warm_ps")
            for _ in range(20):
                nc.tensor.matmul(warm_ps[:, :256], warm_w[:, :128], warm_w[:, :256],
                                 start=True, stop=True)
            warm_pt = cw.tile([128, NH], MMDT, tag="warmpt")
            nc.scalar.activation(warm_pt[:], ones_f[:], EXP, scale=1.0)

            # ---- inputs; x tiles split in column halves across two DMA
            # queues so the first projections start ~3us in ----
            def load_bf16(pool, dram_ap, shape, tag, eng=None):
                r = pool.tile(shape, MMDT, tag=tag, name=f"r_{tag}")
                (eng or nc.sync).dma_start(out=r[:], in_=dram_ap)
                return r

            bcol_sb = cw.tile([128, TT], F32, tag="bcol")
            nc.gpsimd.dma_start(out=bcol_sb[:], in_=bcol[:, :])
            bv_sb = cw.tile([128, DO], F32, tag="bv")
            bv_ap = bv.ap()
            bv_bcast = bass.AP(tensor=bv_ap.tensor, offset=bv_ap.offset, ap=[[0, 128], [1, DO]])
            nc.scalar.dma_start(out=bv_sb[:], in_=bv_bcast)
            wk_r = load_bf16(px, wk[:, :].rearrange("p (kt m) -> p kt m", m=DO), [128, KT, DO], "wk", eng=nc.scalar)
            wq_r = load_bf16(px, wq[:, :].rearrange("p (kt m) -> p kt m", m=DO), [128, KT, DO], "wq", eng=nc.scalar)
            # x tiles arrive in 512-column chunks, nb-major, split across
            # two DMA queues, so the first projection chains finish while
            # the bulk of x is still in flight
            xr = [px.tile([128, T], MMDT, tag=f"xr{kt}", name=f"r_xr{kt}") for kt in range(KT)]
            bq_sb = cw.tile([128, 2], F32, tag="bq")
            nc.gpsimd.dma_start(out=bq_sb[:], in_=bq2[:, :])
            for nb in range(NB):
                csl = slice(nb * 512, (nb + 1) * 512)
                for kt in range(KT):
                    nc.sync.dma_start(out=xr[kt][:, csl],
                                  in_=xt[nb * D + kt * 128:nb * D + (kt + 1) * 128, :])

            # ---- gpsimd queue: biases first (they gate the projection
            # copies), kth zero-halves (gate the first S stationary), wv,
            # then PV-tile zeroing and wo ----
            qt2 = [persist.tile([128, T], MMDT, tag=f"qt{mi}", name=f"qt{mi}") for mi in range(2)]
            kth = [persist.tile([128, T], MMDT, tag=f"kh{h}", name=f"kh{h}") for h in range(NH)]
            vp = persist.tile([128, TT, NH * 128], MMDT, tag="vp")
            at2p = [persist.tile([128, T], MMDT, tag=f"atp{p}", name=f"atp{p}") for p in range(2)]
            for h in H_ORDER:
                lo, hi = ((64, 128) if h % 2 == 0 else (0, 64))
                nc.vector.memset(kth[h][lo:hi, :], 0.0)
            wv_r = load_bf16(px, wv[:, :].rearrange("p (kt m) -> p kt m", m=DO), [128, KT, DO], "wv", eng=nc.gpsimd)
            # PV stationary: [kt, head, 128 cols]; per-head column placement
            # is parity-asymmetric (see module docstring)
            nc.gpsimd.memset(vp[:, 0:4, :], 0.0)
            nc.gpsimd.memset(vp[:, 4:TT, :], 0.0)
            wo_r = load_bf16(px, wo[:, :].rearrange("p (j n) -> p j n", j=2), [128, 2, OUT], "wo", eng=nc.gpsimd)

            # ---- emission helpers (advanced by the interleaver) ----
            # Q/K projection groups can be emitted in two 4-matmul halves
            # (part 0/1) so a filler slot never blocks more than ~0.9us of
            # ready S/PV work behind it in the in-order PE queue.
            halves_open = {}

            def qk_part(which, mi, nb, part, pool_tag="attn"):
                key = (which, mi, nb)
                if part == 0:
                    pool = ps_s if pool_tag == "s" else ps_mm
                    halves_open[key] = pool.tile(
                        [128, 1024], F32, tag=pool_tag, name=f"ps_{which}{mi}_{nb}")
                ps = halves_open[key]
                w_r = wq_r if which == "q" else wk_r
                for kt in range(part * 4, part * 4 + 4):
                    nc.tensor.matmul(
                        ps[:, :512],
                        w_r[:, kt, mi * 128:(mi + 1) * 128],
                        xr[kt][:, nb * 512:(nb + 1) * 512],
                        start=(kt == 0),
                        stop=(kt == KT - 1),
                    )
                if part == 1:
                    del halves_open[key]
                    sl = slice(nb * 512, (nb + 1) * 512)
                    if which == "q":
                        nc.vector.tensor_scalar_add(
                            qt2[mi][:, sl], ps[:, :512], bq_sb[:, mi:mi + 1])
                    else:
                        nc.vector.tensor_scalar_add(
                            kth[2 * mi][0:64, sl], ps[0:64, :512], bq_sb[0:64, mi:mi + 1])
                        nc.vector.tensor_scalar_add(
                            kth[2 * mi + 1][64:128, sl], ps[64:128, :512],
                            bq_sb[64:128, mi:mi + 1])

            def q_group(mi, nb, pool_tag="attn"):
                qk_part("q", mi, nb, 0, pool_tag)
                qk_part("q", mi, nb, 1, pool_tag)

            def k_group(mi, nb, pool_tag="attn"):
                qk_part("k", mi, nb, 0, pool_tag)
                qk_part("k", mi, nb, 1, pool_tag)

            def v_group(tt):
                ps = ps_mm.tile([128, 1024], F32, tag="attn", name=f"ps_v{tt}")
                for kt in range(KT):
                    nc.tensor.matmul(
                        ps[:, :DO],
                        xr[kt][:, tt * 128:(tt + 1) * 128],
                        wv_r[:, kt, :],
                        start=(kt == 0),
                        stop=(kt == KT - 1),
                    )
                vpt = vp[:, tt, :].rearrange("p (h c) -> p h c", c=128)
                # even heads (0,2): V at cols 0:64; odd heads (1,3): cols 64:128
                nc.vector.tensor_tensor(
                    vpt[:, 0::2, 0:64],
                    ps[:, :DO].rearrange("p (h c) -> p h c", c=64)[:, 0::2, :],
                    bv_sb[:, :].rearrange("p (h c) -> p h c", c=64)[:, 0::2, :],
                    ADD,
                )
                nc.vector.tensor_tensor(
                    vpt[:, 1::2, 64:128],
                    ps[:, :DO].rearrange("p (h c) -> p h c", c=64)[:, 1::2, :],
                    bv_sb[:, :].rearrange("p (h c) -> p h c", c=64)[:, 1::2, :],
                    ADD,
                )
                nc.gpsimd.tensor_copy(out=vpt[:, 0::2, 64:65], in_=ones_f[:, 0:2, None])
                nc.gpsimd.tensor_copy(out=vpt[:, 1::2, 0:1], in_=ones_f[:, 2:4, None])

            def emit_pv(h, attn_ps, kt, pt):
                for half in range(2):
                    nc.tensor.matmul(
                        attn_ps[:, half * 512:(half + 1) * 512],
                        vp[:, kt, h * 128:(h + 1) * 128],
                        pt[:, half * 512:(half + 1) * 512],
                        start=(kt == 0),
                        stop=(kt == TT - 1),
                    )

            def normalize(qbp, h, attn_ps, split=False):
                sl = slice(qbp * 1024, (qbp + 1) * 1024)
                if h % 2 == 0:
                    # den at psum p64; move to p0 for recip+broadcast
                    den = cw.tile([65, 1024], F32, tag="den", name=f"den{qbp}_{h}")
                    nc.vector.tensor_copy(out=den[64:65, :], in_=attn_ps[64:65, :])
                    d0 = cw.tile([1, 1024], F32, tag="d0", name=f"d0{qbp}_{h}")
                    nc.sync.dma_start(out=d0[:], in_=den[64:65, :])
                    rec = small.tile([1, 1024], F32, tag="rec", name=f"rec{qbp}_{h}")
                    nc.vector.reciprocal_approx_fast(rec[:], d0[:])
                    rb = small.tile([64, 1024], F32, tag="rb", name=f"rb{qbp}_{h}")
                    nc.gpsimd.partition_broadcast(rb[:], rec[:])
                    nc.vector.tensor_tensor(
                        at2p[h // 2][0:64, sl], attn_ps[0:64, :], rb[:], MULT
                    )
                else:
                    # den already at psum p0: recip straight off psum.
                    # split=True pipelines the two 512-column halves so the
                    # tail output projection can start ~2.5us sooner.
                    rbh = small.tile([128, 1024], F32, tag="rbh", name=f"rbh{qbp}_{h}")
                    halves = ((0, 1024),) if not split else ((0, 512), (512, 1024))
                    for lo, hi in halves:
                        rec = small.tile([1, 1024], F32, tag="rec", name=f"rec{qbp}_{h}_{lo}")
                        nc.vector.reciprocal_approx_fast(rec[:, 0:hi - lo], attn_ps[0:1, lo:hi])
                        nc.gpsimd.partition_broadcast(rbh[:, lo:hi], rec[:, 0:hi - lo])
                        nc.vector.tensor_tensor(
                            at2p[h // 2][64:128, qbp * 1024 + lo:qbp * 1024 + hi],
                            attn_ps[64:128, lo:hi], rbh[64:128, lo:hi], MULT
                        )

            # output projection, one j-partial (head pair) per call; the two
            # partial outputs are summed on the host. This lets half the
            # projection run as soon as its head pair is normalized instead
            # of piling the whole thing after the last head.
            def c_half(j, nt, tb, evac=None, pool=None, dma_eng=None):
                ps = (pool or ps_mm).tile(
                    [128, 1024], F32, tag=("s" if pool is ps_s else "attn"),
                    name=f"ps_c{j}_{nt}_{tb}")
                nc.tensor.matmul(
                    ps[:, :512],
                    wo_r[:, j, nt * 128:(nt + 1) * 128],
                    at2p[j][:, tb * 512:(tb + 1) * 512],
                    start=True,
                    stop=True,
                )
                o_sb = stage.tile([128, 512], F16, tag="stage", name="o_sb")
                if evac == "s":
                    nc.scalar.copy(o_sb[:], ps[:, :512])
                else:
                    nc.vector.tensor_copy(out=o_sb[:], in_=ps[:, :512])
                (dma_eng or nc.sync).dma_start(
                    out=outT[(j * NB + tb) * OUT + nt * 128:(j * NB + tb) * OUT + (nt + 1) * 128, :],
                    in_=o_sb[:],
                )

            # ---- emission schedule ----
            # minimal upfront work for the first unit, then ONE globally
            # software-pipelined stream over all 128 attention units.
            # prep: everything the first S unit needs, plus the remaining
            # K(0,*) groups and early V groups -- all of these trickle at
            # x-chunk-arrival pace inside the DMA wait, where the PE is
            # idle anyway (an in-stream K filler would block ready S work
            # behind it in the in-order PE queue).
            k_group(0, 0, pool_tag="s")
            q_group(0, 0, pool_tag="s")
            q_group(0, 1, pool_tag="s")
            v_group(0)
            v_group(1)
            k_group(0, 1, pool_tag="s")
            v_group(2)
            v_group(3)

            # filler plan: one work item per unit slot. v_group(tt) must be
            # emitted >=1 unit before its PV consumer (at idx tt+1); the
            # K(0,2)/K(0,3) halves go just ahead of the S units that need
            # them (kt 8 and kt 12), where their x chunks have just landed.
            plan = [[] for _ in range(129)]
            vslots = [0, 1, 2, 3, 6, 7, 10, 11, 12, 13, 14, 15]
            for s, tt in zip(vslots, range(4, TT)):
                plan[s].append(lambda tt=tt: v_group(tt))
            plan[4].append(lambda: qk_part("k", 0, 2, 0, "s"))
            plan[5].append(lambda: qk_part("k", 0, 2, 1, "s"))
            plan[8].append(lambda: qk_part("k", 0, 3, 0, "s"))
            plan[9].append(lambda: qk_part("k", 0, 3, 1, "s"))
            qk_jobs = [("k", 1, 0), ("k", 1, 1), ("k", 1, 2), ("k", 1, 3),
                       ("q", 1, 0), ("q", 1, 1)]
            slot = 17
            for which, mi, nb in qk_jobs:
                for part in range(2):
                    plan[slot].append(
                        lambda w=which, m=mi, n=nb, p=part: qk_part(w, m, n, p))
                    slot += 1
            qk_jobs2 = [("q", 0, 2), ("q", 0, 3), ("q", 1, 2), ("q", 1, 3)]
            slot = 34
            for which, mi, nb in qk_jobs2:
                for part in range(2):
                    plan[slot].append(
                        lambda w=which, m=mi, n=nb, p=part: qk_part(w, m, n, p))
                    slot += 1
                slot += 2
            def c_full(nt, tb):
                ps = ps_mm.tile([128, 1024], F32, tag="attn", name=f"ps_cf{nt}_{tb}")
                for j in range(2):
                    nc.tensor.matmul(
                        ps[:, :512],
                        wo_r[:, j, nt * 128:(nt + 1) * 128],
                        at2p[j][:, tb * 512:(tb + 1) * 512],
                        start=(j == 0),
                        stop=(j == 1),
                    )
                o_sb = stage.tile([128, 512], F16, tag="stage", name="o_sb")
                nc.vector.tensor_copy(out=o_sb[:], in_=ps[:, :512])
                nc.sync.dma_start(
                    out=outT[tb * OUT + nt * 128:tb * OUT + (nt + 1) * 128, :],
                    in_=o_sb[:],
                )

            # c jobs: tb0/1 as full j-chains once both pairs of qbp0 are
            # normalized; tb2/3 j-split so only the j=1 partials trail the
            # final normalize
            c_jobs = [(68, "full", (0, 2)), (100, 0, (2, 4))]
            used = {i for i, items in enumerate(plan) if items}
            for ready, j, (tb_lo, tb_hi) in c_jobs:
                jobs = [(j, nt, tb) for tb in range(tb_lo, tb_hi)
                        for nt in range(OUT // 128)]
                s = ready
                while jobs:
                    if s >= 128:
                        raise RuntimeError("c-half jobs did not fit")
                    if s not in used and 1 <= (s % 16) <= 14:
                        jj, nt, tb = jobs.pop(0)
                        if jj == "full":
                            plan[s].append(lambda b=nt, c=tb: c_full(b, c))
                        else:
                            plan[s].append(lambda a=jj, b=nt, c=tb: c_half(a, b, c))
                        used.add(s)
                    s += 1

            units = [(qbp, h, kt) for qbp in range(2) for h in H_ORDER for kt in range(TT)]
            attn_tiles = {}
            prev = None
            for idx, (qbp, h, kt) in enumerate(units):
                if kt == 0:
                    attn_tiles[(qbp, h)] = ps_mm.tile(
                        [128, 1024], F32, tag="attn", name=f"attn_{qbp}_{h}"
                    )
                s_ps = ps_s.tile([128, 1024], F32, tag="s", name=f"s_{qbp}_{h}_{kt}")
                for half in range(2):
                    nc.tensor.matmul(
                        s_ps[:, half * 512:(half + 1) * 512],
                        kth[h][:, kt * 128:(kt + 1) * 128],
                        qt2[h // 2][:, qbp * 1024 + half * 512:qbp * 1024 + (half + 1) * 512],
                        start=True,
                        stop=True,
                    )
                pt = ptp.tile([128, 1024], MMDT, tag="pt")
                nc.scalar.activation(
                    pt[:], s_ps[:], EXP, bias=bcol_sb[:, kt:kt + 1], scale=0.125
                )
                if prev is not None:
                    pq, ph, pk, ppt = prev
                    emit_pv(ph, attn_tiles[(pq, ph)], pk, ppt)
                    if pk == TT - 1:
                        normalize(pq, ph, attn_tiles.pop((pq, ph)))
                for item in plan[idx]:
                    item()
                prev = (qbp, h, kt, pt)
            pq, ph, pk, ppt = prev
            emit_pv(ph, attn_tiles[(pq, ph)], pk, ppt)
            normalize(pq, ph, attn_tiles.pop((pq, ph)), split=True)

            # tail: only the j=1 partials of the last two time blocks remain.
            # Pair two nt blocks per psum tile -> one wide evacuation and one
            # 256KB DMA each; alternate evac engines (ScalarE is idle now).
            for i, (nt, tb) in enumerate([(nt, tb) for tb in range(2, NB) for nt in range(0, OUT // 128, 2)]):
                pool = ps_s if i % 2 == 0 else ps_mm
                ps = pool.tile([128, 1024], F32, tag=("s" if pool is ps_s else "attn"),
                               name=f"ps_ct{nt}_{tb}")
                for two in range(2):
                    nc.tensor.matmul(
                        ps[:, two * 512:(two + 1) * 512],
                        wo_r[:, 1, (nt + two) * 128:(nt + two + 1) * 128],
                        at2p[1][:, tb * 512:(tb + 1) * 512],
                        start=True,
                        stop=True,
                    )
                o_sb = stage.tile([128, 1024], F16, tag="stage2", name="o_sb2")
                if i % 2 == 0:
                    nc.scalar.copy(o_sb[:], ps[:, :1024])
                else:
                    nc.vector.tensor_copy(out=o_sb[:], in_=ps[:, :1024])
                base = (1 * NB + tb) * OUT + nt * 128
                oT_ap = outT.ap()
                dst = bass.AP(tensor=oT_ap.tensor, offset=oT_ap.offset + base * 512,
                              ap=[[512, 128], [128 * 512, 2], [1, 512]])
                (nc.scalar if i % 2 == 0 else nc.sync).dma_start(out=dst, in_=o_sb[:])

            if dbg:
                for mi in range(2):
                    nc.sync.dma_start(out=dbg["d_qt"][mi][:, :], in_=qt2[mi][:])
                for h in range(NH):
                    nc.sync.dma_start(out=dbg["d_kt"][h][:, :], in_=kth[h][:])
                for j in range(2):
                    nc.sync.dma_start(out=dbg["d_at"][j][:, :], in_=at2p[j][:])
                nc.sync.dma_start(out=dbg["d_vp"][:, :, :], in_=vp[:])


def _build():
    nc = bacc.Bacc(trn_type="TRN2")
    build_attention(nc)
    nc.compile()
    return nc


def _get_nc():
    if "nc" not in _CACHE:
        _CACHE["nc"] = _build()
    return _CACHE["nc"]


def make_in_maps(x, W_q, b_q, W_k, W_v, b_v, W_o, bias):
    import ml_dtypes
    bf16 = ml_dtypes.bfloat16

    def warr(w):
        # [D, DO] -> SBUF layout [128, KT*DO] (partition-major, kt-tiled)
        return np.ascontiguousarray(
            w.reshape(KT, 128, DO).transpose(1, 0, 2).reshape(128, KT * DO))

    def woarr(w):
        # [2*128, OUT] -> [two*64+p, j, n] -> [128, 2*OUT]
        return np.ascontiguousarray(
            w.reshape(2, 2, 64, OUT).transpose(1, 2, 0, 3).reshape(128, 2 * OUT))

    in_maps = []
    xtb = [np.ascontiguousarray(
        x[b].T.astype(bf16).reshape(D, NB, 512).transpose(1, 0, 2).reshape(NB * D, 512))
        for b in range(B)]
    wqb = W_q.astype(bf16)
    wkb = W_k.astype(bf16)
    wvb = W_v.astype(bf16)
    wob = W_o.astype(bf16)
    for c in range(8):
        b, hg = divmod(c, 4)
        sl = slice(hg * DO, (hg + 1) * DO)
        in_maps.append({
            "xt": xtb[b],
            "wq": warr(wqb[:, sl]),
            "wk": warr(wkb[:, sl]),
            "wv": warr(wvb[:, sl]),
            "wo": woarr(wob[sl, :]),
            "bq2": np.ascontiguousarray(b_q[sl].reshape(2, 128).T),
            "bv": np.ascontiguousarray(b_v[sl]),
            "bcol": np.ascontiguousarray(bias.reshape(TT, 128).T),
        })
    return in_maps


def kernel(x, W_q, b_q, W_k, b_k, W_v, b_v, W_o, b_o, bias, **_ignored):
    x = np.asarray(x, dtype=np.float32)
    W_q = np.asarray(W_q, dtype=np.float32)
    W_k = np.asarray(W_k, dtype=np.float32)
    W_v = np.asarray(W_v, dtype=np.float32)
    W_o = np.asarray(W_o, dtype=np.float32)
    b_q = np.asarray(b_q, dtype=np.float32)
    b_v = np.asarray(b_v, dtype=np.float32)
    b_o = np.asarray(b_o, dtype=np.float32)
    bias = np.asarray(bias, dtype=np.float32)

    nc = _get_nc()
    in_maps = make_in_maps(x, W_q, b_q, W_k, W_v, b_v, W_o, bias)
    _CACHE["in_maps"] = in_maps
    res = run_bass_kernel_spmd(nc, in_maps, list(range(8)))
    out = np.zeros((B, T, OUT), dtype=np.float32)
    for c in range(8):
        oc = res.results[c]["outT"].reshape(2, NB, OUT, 512).astype(np.float32)
        parts = [oc[0, 0], oc[0, 1], oc[0, 2] + oc[1, 2], oc[0, 3] + oc[1, 3]]
        out[c // 4] += np.concatenate(parts, axis=1).T
    out += b_o
    return out



# revision 13
# speedup vs baseline: 1.1085x; 1.1085x over previous
"""Multi-head attention (B=2, T=2048, D=OUT=1024, H=16) on 8 TRN2 NeuronCores.

Sharding: data-parallel over batch (2 groups) x tensor-parallel over heads
(4 groups of 4 heads). Core c handles batch c//4, heads (c%4)*4..(c%4)*4+4.

Structure (v2, transposed-PV):
- Projections as before: Q^T/K^T tiles ([pair-depth, T], keys zero-padded per
  parity), V blocks with keys on partitions.
- S = kth^T qt per (head, key-tile): [128 keys, 1024 q] psum; exp on ACT with
  scale only -- the additive position bias (per key) is folded into V and the
  denominator column as exp(bias) (softmax identity: exp(s+b) = exp(s)e^b),
  which removes the bias operand from the activation (~220ns/instr cheaper).
- PV is TRANSPOSED: stationary = pt[:, qb*128:+128] (exp scores), moving =
  V' tile [128 keys, 65] (64 V cols scaled by e^bias + one e^bias column that
  accumulates the softmax denominator). Output psum [128 q, 65] per
  (head, q-block) chains over 16 key tiles. This halves PV matmul columns vs
  the padded [depth|ones] layout (65 vs 128 output partitions used per col,
  i.e. moving dim is 65 instead of 1024-queries x2 passes).
- Normalize is per-partition: reciprocal of the den column [128,1], then one
  tensor_scalar multiply into the attn staging tile. No partition broadcasts
  or cross-partition DMA hops.
- A PE transpose (identity moving, 56ns) flips each normalized [128 q, 128 c]
  head-pair block into the [c, T] layout the output projection consumes.
- Output projection is fully j-split (one 128-contraction matmul per
  (j, nt, tb)); the host sums the two j-partials for every time block.
- x is loaded with ONE dma_start per 512-query block ([128, 8, 512] strided
  AP) -- dma issues occupy the issuing queue ~800ns each, so fewer/bigger
  issues pace inbound data much faster.
- Emission: 128 S-units with a slot plan; PV chains of the previous head run
  2..9 slots into the next head-group, transposes two slots after the pair
  completes, projection groups placed against their S-deadlines, output
  projection spread over the last ~80 slots, short dense tail.
"""

import numpy as np

import concourse.bass as bass
import concourse.mybir as mybir
import concourse.tile as tile
from concourse import bacc
from concourse.bass_utils import run_bass_kernel_spmd

B, T, D, OUT, H = 2, 2048, 1024, 1024, 16
DO = 256            # output columns per core (4 heads x 64)
DEPTH = 64
NH = 4              # heads per core
KT = D // 128       # 8 contraction tiles for the projections
TT = T // 128       # 16 key tiles
NB = T // 512       # 4 query/time blocks
F32 = mybir.dt.float32
F16 = mybir.dt.float16
BF16 = mybir.dt.bfloat16
MMDT = BF16
EXP = mybir.ActivationFunctionType.Exp
MULT = mybir.AluOpType.mult
ADD = mybir.AluOpType.add

H_ORDER = [1, 0, 2, 3]   # emission order of heads within a query-block pass
POS = H_ORDER + H_ORDER  # head by position p (0..7); qbp = p//4

_CACHE = {}


def build_attention(nc, dbg=False):
    if dbg:
        d_qt = [nc.declare_dram_parameter(f"d_qt{mi}", [128, T], MMDT, isOutput=True) for mi in range(2)]
        d_kt = [nc.declare_dram_parameter(f"d_kt{h}", [128, T], MMDT, isOutput=True) for h in range(NH)]
        d_vp = nc.declare_dram_parameter("d_vp", [128, TT, NH * 65], MMDT, isOutput=True)
        d_at = [nc.declare_dram_parameter(f"d_at{j}", [128, T], MMDT, isOutput=True) for j in range(2)]
        d_a2 = [nc.declare_dram_parameter(f"d_a2_{qbp}_{j}", [128, 8 * 128], MMDT, isOutput=True)
                for qbp in range(2) for j in range(2)]
        d_pt = nc.declare_dram_parameter("d_pt", [128, 1024], MMDT, isOutput=True)
    xt = nc.declare_dram_parameter("xt", [NB * D, 512], MMDT, isOutput=False)
    wq = nc.declare_dram_parameter("wq", [128, KT * DO], MMDT, isOutput=False)
    wk = nc.declare_dram_parameter("wk", [128, KT * DO], MMDT, isOutput=False)
    wv = nc.declare_dram_parameter("wv", [128, KT * DO], MMDT, isOutput=False)
    wo = nc.declare_dram_parameter("wo", [128, 2 * OUT], MMDT, isOutput=False)
    bq2 = nc.declare_dram_parameter("bq2", [128, 2], F32, isOutput=False)
    bv = nc.declare_dram_parameter("bv", [DO], F32, isOutput=False)
    ebias = nc.declare_dram_parameter("ebias", [128, TT], F32, isOutput=False)
    ident = nc.declare_dram_parameter("ident", [128, 128], MMDT, isOutput=False)
    outT = nc.declare_dram_parameter("outT", [2 * NB * OUT, 512], F16, isOutput=True)

    with tile.TileContext(nc) as tc:
        with (
            tc.tile_pool(name="cw", bufs=1) as cw,
            tc.tile_pool(name="stage", bufs=8) as stage,
            tc.tile_pool(name="persist", bufs=1) as persist,
            tc.tile_pool(name="small", bufs=8) as small,
            tc.tile_pool(name="vtp", bufs=2) as vtp,
            tc.tile_pool(name="ptp", bufs=28) as ptp,
            tc.tile_pool(name="px", bufs=1) as px,
            tc.tile_pool(name="ps_s", bufs=2, space="PSUM") as ps_s,
            tc.tile_pool(name="ps_f", bufs=2, space="PSUM") as ps_f,
            tc.tile_pool(name="ps_pv", bufs=1, space="PSUM") as ps_pv,
            tc.tile_pool(name="ps_t", bufs=1, space="PSUM") as ps_t,
        ):
            # ---- warmup: wake the PE HAM clock gate and the ACT exp table
            # while the input DMAs are in flight ----
            ones_f = cw.tile([128, NH], F32, tag="ones")
            nc.vector.memset(ones_f[:], 1.0)
            warm_w = cw.tile([128, 256], MMDT, tag="warmw")
            nc.vector.memset(warm_w[:], 0.0)
            warm_ps = ps_s.tile([128, 256], F32, tag="s", name="warm_ps")
            for _ in range(20):
                nc.tensor.matmul(warm_ps[:, :256], warm_w[:, :128], warm_w[:, :256],
                                 start=True, stop=True)
            warm_pt = cw.tile([128, NH], MMDT, tag="warmpt")
            nc.scalar.activation(warm_pt[:], ones_f[:], EXP, scale=1.0)

            # ---- inputs ----
            # queue split: sync = x (one big issue per nb); scalar = wk,wq
            # (idle before the first exp); gpsimd = the small tensors + wv,wo.
            id_sb = cw.tile([128, 128], MMDT, tag="id")
            nc.gpsimd.dma_start(out=id_sb[:], in_=ident[:, :])
            eb_sb = cw.tile([128, TT], F32, tag="eb")
            nc.gpsimd.dma_start(out=eb_sb[:], in_=ebias[:, :])
            bq_sb = cw.tile([128, 2], F32, tag="bq")
            nc.gpsimd.dma_start(out=bq_sb[:], in_=bq2[:, :])
            bv_sb = cw.tile([128, DO], F32, tag="bv")
            bv_ap = bv.ap()
            bv_bcast = bass.AP(tensor=bv_ap.tensor, offset=bv_ap.offset,
                               ap=[[0, 128], [1, DO]])
            nc.gpsimd.dma_start(out=bv_sb[:], in_=bv_bcast)

            def load_bf16(dram_ap, shape, tag, eng):
                r = px.tile(shape, MMDT, tag=tag, name=f"r_{tag}")
                eng.dma_start(out=r[:], in_=dram_ap)
                return r

            wk_r = load_bf16(wk[:, :].rearrange("p (kt m) -> p kt m", m=DO),
                             [128, KT, DO], "wk", nc.scalar)
            wq_r = load_bf16(wq[:, :].rearrange("p (kt m) -> p kt m", m=DO),
                             [128, KT, DO], "wq", nc.scalar)
            wv_r = load_bf16(wv[:, :].rearrange("p (kt m) -> p kt m", m=DO),
                             [128, KT, DO], "wv", nc.gpsimd)
            wo_r = load_bf16(wo[:, :].rearrange("p (j n) -> p j n", j=2),
                             [128, 2, OUT], "wo", nc.gpsimd)

            # x: one [128, KT, 512] strided transfer per query block
            x_all = px.tile([128, KT, T], MMDT, tag="xall")
            xt_ap = xt.ap()
            for nb in range(NB):
                src = bass.AP(tensor=xt_ap.tensor, offset=xt_ap.offset + nb * D * 512,
                              ap=[[512, 128], [128 * 512, KT], [1, 512]])
                nc.sync.dma_start(out=x_all[:, :, nb * 512:(nb + 1) * 512], in_=src)

            # ---- persistent tiles ----
            qt2 = [persist.tile([128, T], MMDT, tag=f"qt{mi}", name=f"qt{mi}") for mi in range(2)]
            kth = [persist.tile([128, T], MMDT, tag=f"kh{h}", name=f"kh{h}") for h in range(NH)]
            vp = persist.tile([128, TT, NH * 65], MMDT, tag="vp")
            at2p = [persist.tile([128, T], MMDT, tag=f"atp{j}", name=f"atp{j}") for j in range(2)]
            attn2 = [[persist.tile([128, 8 * 128], MMDT, tag=f"a2_{qbp}_{j}",
                                   name=f"a2_{qbp}_{j}") for j in range(2)] for qbp in range(2)]
            for h in H_ORDER:
                lo, hi = ((64, 128) if h % 2 == 0 else (0, 64))
                nc.vector.memset(kth[h][lo:hi, :], 0.0)
            # e^bias columns of the V' tiles, all (tt, h) in one strided copy
            eba = eb_sb[:]
            eb_bcast = bass.AP(tensor=eba.tensor, offset=eba.offset,
                               ap=[eba.ap[0], eba.ap[1], [0, NH]])
            vp_cols = vp[:, :, :].rearrange("p t (h c) -> p t h c", c=65)[:, :, :, 64]
            nc.gpsimd.tensor_copy(out=vp_cols, in_=eb_bcast)

            # ---- helpers ----
            halves_open = {}

            def qk_part(which, mi, nb, part):
                key = (which, mi, nb)
                if part == 0:
                    halves_open[key] = ps_f.tile(
                        [128, 512], F32, tag="f", name=f"ps_{which}{mi}_{nb}")
                ps = halves_open[key]
                w_r = wq_r if which == "q" else wk_r
                for kt in range(part * 4, part * 4 + 4):
                    nc.tensor.matmul(
                        ps[:, :],
                        w_r[:, kt, mi * 128:(mi + 1) * 128],
                        x_all[:, kt, nb * 512:(nb + 1) * 512],
                        start=(kt == 0),
                        stop=(kt == KT - 1),
                    )
                if part == 1:
                    del halves_open[key]
                    sl = slice(nb * 512, (nb + 1) * 512)
                    if which == "q":
                        nc.vector.tensor_scalar_add(
                            qt2[mi][:, sl], ps[:, :], bq_sb[:, mi:mi + 1])
                    else:
                        nc.vector.tensor_scalar_add(
                            kth[2 * mi][0:64, sl], ps[0:64, :], bq_sb[0:64, mi:mi + 1])
                        nc.vector.tensor_scalar_add(
                            kth[2 * mi + 1][64:128, sl], ps[64:128, :],
                            bq_sb[64:128, mi:mi + 1])

            def q_group(mi, nb):
                qk_part("q", mi, nb, 0)
                qk_part("q", mi, nb, 1)

            def k_group(mi, nb):
                qk_part("k", mi, nb, 0)
                qk_part("k", mi, nb, 1)

            def v_group(tt):
                ps = ps_f.tile([128, 512], F32, tag="f", name=f"ps_v{tt}")
                for kt in range(KT):
                    nc.tensor.matmul(
                        ps[:, :DO],
                        x_all[:, kt, tt * 128:(tt + 1) * 128],
                        wv_r[:, kt, :],
                        start=(kt == 0),
                        stop=(kt == KT - 1),
                    )
                tmp = vtp.tile([128, DO], F32, tag="vtmp", name=f"vtmp{tt}")
                nc.vector.tensor_tensor(tmp[:], ps[:, :DO], bv_sb[:], ADD)
                vpt = vp[:, tt, :].rearrange("p (h c) -> p h c", c=65)
                nc.gpsimd.tensor_scalar_mul(
                    vpt[:, :, 0:64],
                    tmp[:].rearrange("p (h c) -> p h c", c=64),
                    eb_sb[:, tt:tt + 1])

            pv_ps = ps_pv.tile([128, 4, 65], F32, tag="pv", name="pv_ps")
            tps = ps_t.tile([128, 2, 128], MMDT, tag="tp", name="tps")
            pt_tiles = {}
            tcount = [0]
            ccount = [0]

            def pv_chain(qbp, h, qb):
                slot = qb % 4
                for kt in range(TT):
                    nc.tensor.matmul(
                        pv_ps[:, slot, :],
                        pt_tiles[(qbp, h, kt)][:, qb * 128:(qb + 1) * 128],
                        vp[:, kt, h * 65:(h + 1) * 65],
                        start=(kt == 0),
                        stop=(kt == TT - 1),
                    )
                # after every 4th chain: one batched reciprocal of the four
                # denominator columns, then the four normalize multiplies
                if slot == 3:
                    rec4 = small.tile([128, 4], F32, tag="rec", name=f"rec{qbp}_{h}_{qb}")
                    nc.vector.reciprocal_approx_fast(rec4[:], pv_ps[:, 0:4, 64])
                    j, par = h // 2, h % 2
                    for s in range(4):
                        qq = qb - 3 + s
                        nc.vector.tensor_scalar_mul(
                            attn2[qbp][j][:, qq * 128 + par * 64:qq * 128 + par * 64 + 64],
                            pv_ps[:, s, 0:64], rec4[:, s:s + 1])

            def transp(qbp, j, qb):
                ts = tcount[0] % 2
                tcount[0] += 1
                nc.tensor.transpose(tps[:, ts, :],
                                    attn2[qbp][j][:, qb * 128:(qb + 1) * 128], id_sb[:])
                nc.vector.tensor_copy(
                    out=at2p[j][:, qbp * 1024 + qb * 128:qbp * 1024 + (qb + 1) * 128],
                    in_=tps[:, ts, :])

            def c_half(j, nt, tb, evac=None):
                ps = ps_f.tile([128, 512], F32, tag="f", name=f"ps_c{j}_{nt}_{tb}")
                if j == "full":
                    for jj in range(2):
                        nc.tensor.matmul(
                            ps[:],
                            wo_r[:, jj, nt * 128:(nt + 1) * 128],
                            at2p[jj][:, tb * 512:(tb + 1) * 512],
                            start=(jj == 0),
                            stop=(jj == 1),
                        )
                    j = 0
                else:
                    nc.tensor.matmul(
                        ps[:],
                        wo_r[:, j, nt * 128:(nt + 1) * 128],
                        at2p[j][:, tb * 512:(tb + 1) * 512],
                        start=True,
                        stop=True,
                    )
                o_sb = stage.tile([128, 512], F16, tag="stage", name="o_sb")
                if evac == "s":
                    nc.scalar.copy(o_sb[:], ps[:])
                else:
                    nc.vector.tensor_copy(out=o_sb[:], in_=ps[:])
                nc.sync.dma_start(
                    out=outT[(j * NB + tb) * OUT + nt * 128:(j * NB + tb) * OUT + (nt + 1) * 128, :],
                    in_=o_sb[:],
                )

            # ---- prep: everything the first S unit needs, plus early V ----
            k_group(0, 0)
            q_group(0, 0)
            q_group(0, 1)
            v_group(0)
            v_group(1)
            k_group(0, 1)
            v_group(2)
            v_group(3)

            # ---- slot plan ----
            plan = [[] for _ in range(128)]

            def put(s, fn):
                plan[s].append(fn)

            # v_groups at x-arrival pace
            vslots = [0, 1, 2, 3, 6, 7, 10, 11, 12, 13, 14, 15]
            for s, tt in zip(vslots, range(4, TT)):
                put(s, lambda tt=tt: v_group(tt))
            # K(0,2)/(0,3) just before the S units that need them
            put(4, lambda: qk_part("k", 0, 2, 0))
            put(5, lambda: qk_part("k", 0, 2, 1))
            put(8, lambda: qk_part("k", 0, 3, 0))
            put(9, lambda: qk_part("k", 0, 3, 1))
            # PV chains of the previous head: pos p (1..7), slots 16p+2+qb
            for p in range(1, 8):
                for qb in range(8):
                    put(16 * p + 2 + qb,
                        lambda p=p, qb=qb: pv_chain((p - 1) // 4, POS[p - 1], qb))
            # transposes after the pair's second-head normalize batches land
            # (batch for qb0-3 at rel slot +5, qb4-7 at +9 of the next group)
            for sp, qbp, j in [(1, 0, 0), (3, 0, 1), (5, 1, 0)]:
                for qb in range(8):
                    put(16 * (sp + 1) + 6 + qb,
                        lambda qbp=qbp, j=j, qb=qb: transp(qbp, j, qb))
            # projection groups against their S deadlines (hand layout)
            qk_sched = [
                (16, "q", 1, 0, 0), (17, "q", 1, 0, 1),
                (26, "q", 1, 1, 0), (27, "q", 1, 1, 1),
                (28, "k", 1, 0, 0), (29, "k", 1, 0, 1),
                (30, "k", 1, 1, 0), (31, "k", 1, 1, 1),
                (32, "k", 1, 2, 0), (33, "k", 1, 2, 1),
                (34, "k", 1, 3, 0), (35, "k", 1, 3, 1),
                (44, "q", 0, 2, 0), (45, "q", 0, 2, 1),
                (46, "q", 0, 3, 0), (47, "q", 0, 3, 1),
                (48, "q", 1, 2, 0), (49, "q", 1, 2, 1),
                (52, "q", 1, 3, 0), (53, "q", 1, 3, 1),
            ]
            for s, w, m, n, part in qk_sched:
                put(s, lambda w=w, m=m, n=n, p=part: qk_part(w, m, n, p))
            # output projection jobs, greedy from their ready slots: full
            # (both-j) sums for tb0/1, j-split for tb2/3 (j=1 trails in the tail)
            c_jobs = [(75, "full", 0), (79, "full", 1), (107, 0, 2), (111, 0, 3)]
            for ready, j, tb in c_jobs:
                s = ready
                for nt in range(OUT // 128):
                    while s < 128 and len(plan[s]) >= 2:
                        s += 1
                    if s >= 128:
                        raise RuntimeError("c jobs did not fit")
                    put(s, lambda j=j, nt=nt, tb=tb: c_half(j, nt, tb))
                    s += 1

            # ---- main stream: 128 S units ----
            units = [(p // 4, POS[p], kt) for p in range(8) for kt in range(TT)]
            for idx, (qbp, h, kt) in enumerate(units):
                s_ps = ps_s.tile([128, 1024], F32, tag="s", name=f"s_{idx}")
                for half in range(2):
                    nc.tensor.matmul(
                        s_ps[:, half * 512:(half + 1) * 512],
                        kth[h][:, kt * 128:(kt + 1) * 128],
                        qt2[h // 2][:, qbp * 1024 + half * 512:qbp * 1024 + (half + 1) * 512],
                        start=True,
                        stop=True,
                    )
                pt = ptp.tile([128, 1024], MMDT, tag="pt", name=f"pt{idx}")
                nc.scalar.activation(pt[:], s_ps[:], EXP, scale=0.125)
                pt_tiles[(qbp, h, kt)] = pt
                if dbg and idx == 0:
                    nc.sync.dma_start(out=d_pt[:, :], in_=pt[:])
                for item in plan[idx]:
                    item()

            # ---- tail: last head's PV, j=1 transposes, j=1 out-proj tb2/3 ----
            last_h = POS[7]
            for qb in range(4):
                pv_chain(1, last_h, qb)
            for qb in range(4, 8):
                pv_chain(1, last_h, qb)
                transp(1, 1, qb - 4)
            transp(1, 1, 3)
            for nt in range(0, 4):
                c_half(1, nt, 2, evac="s")
            for qb in range(4, 8):
                transp(1, 1, qb)
            for nt in range(4, 8):
                c_half(1, nt, 2, evac="s")
            for nt in range(8):
                c_half(1, nt, 3, evac="s")

            if dbg:
                for mi in range(2):
                    nc.sync.dma_start(out=d_qt[mi][:, :], in_=qt2[mi][:])
                for h in range(NH):
                    nc.sync.dma_start(out=d_kt[h][:, :], in_=kth[h][:])
                nc.sync.dma_start(out=d_vp[:, :, :], in_=vp[:])
                for j in range(2):
                    nc.sync.dma_start(out=d_at[j][:, :], in_=at2p[j][:])
                for qbp in range(2):
                    for j in range(2):
                        nc.sync.dma_start(out=d_a2[qbp * 2 + j][:, :], in_=attn2[qbp][j][:])



def _build():
    nc = bacc.Bacc(trn_type="TRN2")
    build_attention(nc)
    nc.compile()
    return nc


def _get_nc():
    if "nc" not in _CACHE:
        _CACHE["nc"] = _build()
    return _CACHE["nc"]


def make_in_maps(x, W_q, b_q, W_k, W_v, b_v, W_o, bias):
    import ml_dtypes
    bf16 = ml_dtypes.bfloat16

    def warr(w):
        # [D, DO] -> SBUF layout [128, KT*DO] (partition-major, kt-tiled)
        return np.ascontiguousarray(
            w.reshape(KT, 128, DO).transpose(1, 0, 2).reshape(128, KT * DO))

    def woarr(w):
        # [2*128, OUT] -> [two*64+p, j, n] -> [128, 2*OUT]
        return np.ascontiguousarray(
            w.reshape(2, 2, 64, OUT).transpose(1, 2, 0, 3).reshape(128, 2 * OUT))

    in_maps = []
    xtb = [np.ascontiguousarray(
        x[b].T.astype(bf16).reshape(D, NB, 512).transpose(1, 0, 2).reshape(NB * D, 512))
        for b in range(B)]
    wqb = W_q.astype(bf16)
    wkb = W_k.astype(bf16)
    wvb = W_v.astype(bf16)
    wob = W_o.astype(bf16)
    ebias = np.ascontiguousarray(np.exp(bias.astype(np.float64)).astype(np.float32)
                                 .reshape(TT, 128).T)
    ident = np.eye(128, dtype=np.float32).astype(bf16)
    for c in range(8):
        b, hg = divmod(c, 4)
        sl = slice(hg * DO, (hg + 1) * DO)
        in_maps.append({
            "xt": xtb[b],
            "wq": warr(wqb[:, sl]),
            "wk": warr(wkb[:, sl]),
            "wv": warr(wvb[:, sl]),
            "wo": woarr(wob[sl, :]),
            "bq2": np.ascontiguousarray(b_q[sl].reshape(2, 128).T),
            "bv": np.ascontiguousarray(b_v[sl]),
            "ebias": ebias,
            "ident": ident,
        })
    return in_maps


def kernel(x, W_q, b_q, W_k, b_k, W_v, b_v, W_o, b_o, bias, **_ignored):
    x = np.asarray(x, dtype=np.float32)
    W_q = np.asarray(W_q, dtype=np.float32)
    W_k = np.asarray(W_k, dtype=np.float32)
    W_v = np.asarray(W_v, dtype=np.float32)
    W_o = np.asarray(W_o, dtype=np.float32)
    b_q = np.asarray(b_q, dtype=np.float32)
    b_v = np.asarray(b_v, dtype=np.float32)
    b_o = np.asarray(b_o, dtype=np.float32)
    bias = np.asarray(bias, dtype=np.float32)

    nc = _get_nc()
    in_maps = make_in_maps(x, W_q, b_q, W_k, W_v, b_v, W_o, bias)
    _CACHE["in_maps"] = in_maps
    res = run_bass_kernel_spmd(nc, in_maps, list(range(8)))
    out = np.zeros((B, T, OUT), dtype=np.float32)
    for c in range(8):
        oc = res.results[c]["outT"].reshape(2, NB, OUT, 512).astype(np.float32)
        parts = [oc[0, 0], oc[0, 1], oc[0, 2] + oc[1, 2], oc[0, 3] + oc[1, 3]]
        out[c // 4] += np.concatenate(parts, axis=1).T
    out += b_o
    return out
